# revision 11
# baseline (speedup 1.0000x reference)
"""Trainium2 Bass kernel for nn_Discriminator (down-projection + 16-step LSTM + head).

Computation (per reference):
    x: [512, 16, 10001] fp32
    xa = x[:, :, :10000] @ W_down                      # [B, T, 128]
    xc_t = concat([xa_t, xw_t], -1)                    # per step, [B, 129]
    LSTM over T=16 steps, H=512, forget bias +1:
        gates = [xc_t, h] @ W_cell + b_cell            # [B, 2048] = [i|c|f|o]
        c = c*sig(f+1) + sig(i)*tanh(c_)
        h = sig(o)*tanh(c)
    pred = h @ W_out + b_out                           # [B, 1]

Sharding: pure data-parallel over batch, 64 rows/core on 8 cores. No collectives.

Per-core layout choices:
  - x is pre-transposed on host to xT [10112(pad), 1024] with column index
    t*64+b (t-major) so the PE contraction dim (n) is on partitions, and the
    down-projection directly produces xa^T [128, 1024] whose per-step slice
    [128, 64] is the lhsT of the LSTM's xa matmul.
  - LSTM gates are computed in PSUM [128, 1024]: partitions 0:64 hold the
    [i|c] gate columns, partitions 64:128 hold [f|o]; the two halves are fed
    by concurrent PE column-group matmuls (tile col-tiling), so no extra adds.
  - sigmoid(i) and sigmoid(f+1) are fused into a single [128, 512] activation
    with a per-partition bias of (0 x64, 1 x64).
  - h is transposed back each step via PE-transpose (4x [64,128] tiles) to
    form the next step's lhsT.
"""

import numpy as np
from contextlib import ExitStack

NCORES = 8
B = 512
BC = B // NCORES          # 64 batch rows per core
T = 16
BT = BC * T               # 1024
N = 10000
KT = 79                   # ceil(10000/128)
NPAD = KT * 128           # 10112
HIN = 128
H = 512
G4 = 4 * H                # 2048
DP_CHUNKS = 2             # down-projection output chunks (t-blocks)
CCOLS = BT // DP_CHUNKS   # columns per chunk (512)

# Compute dtype for matmul operands: "float32r" (full-rate fp32 PE mode) or
# "bfloat16" (halves HBM traffic for x; small accuracy cost).
MM_DTYPE = "bfloat16"

_CACHE = {}


def _np_mm_dtype():
    if MM_DTYPE == "bfloat16":
        import ml_dtypes
        return ml_dtypes.bfloat16
    return np.float32


def _build_module():
    import concourse.bass as bass  # noqa: F401
    import concourse.bacc as bacc
    import concourse.tile as tile
    import concourse.mybir as mybir

    AF = mybir.ActivationFunctionType
    f32 = mybir.dt.float32
    mmdt = mybir.dt.bfloat16 if MM_DTYPE == "bfloat16" else mybir.dt.float32r

    nc = bacc.Bacc("TRN2")

    xT = nc.declare_dram_parameter("xT", [NPAD, BT], mmdt, isOutput=False)
    xw = nc.declare_dram_parameter("xw", [2, BT], mmdt, isOutput=False)
    Wd = nc.declare_dram_parameter("Wd", [128, KT * 128], mmdt, isOutput=False)
    Wxa = nc.declare_dram_parameter("Wxa", [128, G4], mmdt, isOutput=False)
    Wxwb = nc.declare_dram_parameter("Wxwb", [2, G4], mmdt, isOutput=False)
    Wh = nc.declare_dram_parameter("Wh", [128, 4 * G4], mmdt, isOutput=False)
    Wo = nc.declare_dram_parameter("Wo", [128, 4], mmdt, isOutput=False)
    bout = nc.declare_dram_parameter("bout", [BC, 1], f32, isOutput=False)
    ident = nc.declare_dram_parameter("ident", [BC, BC], mmdt, isOutput=False)
    pred = nc.declare_dram_parameter("pred", [BC, 1], f32, isOutput=True)

    with ExitStack() as ctx:
        tc = ctx.enter_context(tile.TileContext(nc))
        singles = ctx.enter_context(tc.tile_pool(name="singles", bufs=1))
        slabs = ctx.enter_context(tc.tile_pool(name="slabs", bufs=8))
        work = ctx.enter_context(tc.tile_pool(name="work", bufs=2))
        state = ctx.enter_context(tc.tile_pool(name="state", bufs=2))
        dpp = ctx.enter_context(tc.tile_pool(name="dpp", bufs=2, space="PSUM"))
        gp = ctx.enter_context(tc.tile_pool(name="gp", bufs=2, space="PSUM"))
        tp = ctx.enter_context(tc.tile_pool(name="tp", bufs=2, space="PSUM"))

        # Resident weights / small tensors
        Wd_sb = singles.tile([128, KT * 128], mmdt)
        nc.sync.dma_start(Wd_sb[:], Wd[:])
        Wxa_sb = singles.tile([128, G4], mmdt)
        nc.sync.dma_start(Wxa_sb[:], Wxa[:])
        Wxwb_sb = singles.tile([2, G4], mmdt)
        nc.sync.dma_start(Wxwb_sb[:], Wxwb[:])
        Wh_sb = singles.tile([128, 4 * G4], mmdt)
        nc.sync.dma_start(Wh_sb[:], Wh[:])
        Wo_sb = singles.tile([128, 4], mmdt)
        nc.sync.dma_start(Wo_sb[:], Wo[:])
        xw_sb = singles.tile([2, BT], mmdt)
        nc.sync.dma_start(xw_sb[:], xw[:])
        bout_sb = singles.tile([BC, 1], f32)
        nc.sync.dma_start(bout_sb[:], bout[:])
        id_sb = singles.tile([BC, BC], mmdt)
        nc.sync.dma_start(id_sb[:], ident[:])

        # xa^T, one tile per chunk so the LSTM's dependency is per-chunk
        xaT_sb = [singles.tile([128, CCOLS], mmdt, name=f"xaT{c}") for c in range(DP_CHUNKS)]

        hT_prev = None
        c_prev = None

        def lstm_step(t):
            nonlocal hT_prev, c_prev
            g = gp.tile([128, 2 * H], mybir.dt.float32, tag="g")
            ktiles = [
                (xaT_sb[t * BC // CCOLS][:, (t * BC) % CCOLS:(t * BC) % CCOLS + BC], Wxa_sb),
                (xw_sb[:, t * BC:(t + 1) * BC], Wxwb_sb),
            ]
            if hT_prev is not None:
                for k in range(4):
                    ktiles.append((hT_prev[:, k, :], Wh_sb[:, k * G4:(k + 1) * G4]))
            nk = len(ktiles)
            for ki, (lh, rh) in enumerate(ktiles):
                st, sp = ki == 0, ki == nk - 1
                # halves: 0 -> psum partitions 0:64 accumulate gate cols [i|c],
                #         1 -> partitions 64:128 accumulate [f|o].
                # Emit (half0, half1) adjacent per 512-chunk so the two PE
                # column-groups stream concurrently.
                for ch in range(2):
                    for half in range(2):
                        outap = g[half * 64:(half + 1) * 64, ch * H:(ch + 1) * H]
                        rhap = rh[:, half * 2 * H + ch * H: half * 2 * H + (ch + 1) * H]
                        nc.tensor.matmul(outap, lh, rhap, start=st, stop=sp)

            # activations (all outputs are base-partition-0 tiles: the DVE
            # requires matching base partitions on two-SBUF-input ops)
            sig_i = work.tile([BC, H], mybir.dt.float32, tag="si")
            nc.scalar.activation(sig_i[:], g[0:64, 0:H], AF.Sigmoid)
            sig_f = work.tile([BC, H], mybir.dt.float32, tag="sf")
            nc.scalar.activation(sig_f[:], g[64:128, 0:H], AF.Sigmoid, bias=1.0)
            tanh_c = work.tile([BC, H], mybir.dt.float32, tag="tc")
            nc.scalar.activation(tanh_c[:], g[0:64, H:2 * H], AF.Tanh)
            sig_o = work.tile([BC, H], mybir.dt.float32, tag="so")
            nc.scalar.activation(sig_o[:], g[64:128, H:2 * H], AF.Sigmoid)

            c_new = state.tile([BC, H], mybir.dt.float32, tag="c")
            if c_prev is None:
                nc.vector.tensor_mul(c_new[:], sig_i[:], tanh_c[:])
            else:
                m1 = work.tile([BC, H], mybir.dt.float32, tag="m1")
                nc.vector.tensor_mul(m1[:], c_prev[:], sig_f[:])
                m2 = work.tile([BC, H], mybir.dt.float32, tag="m2")
                nc.vector.tensor_mul(m2[:], sig_i[:], tanh_c[:])
                nc.vector.tensor_add(c_new[:], m1[:], m2[:])
            c_prev = c_new

            tanh_cn = work.tile([BC, H], mybir.dt.float32, tag="tcn")
            nc.scalar.activation(tanh_cn[:], c_new[:], AF.Tanh)
            h = work.tile([BC, H], mmdt, tag="h")
            nc.vector.tensor_mul(h[:], sig_o[:], tanh_cn[:])

            hT = state.tile([128, 4, BC], mmdt, tag="hT")
            for k in range(4):
                tps = tp.tile([128, BC], mmdt, tag="tp")
                nc.tensor.transpose(tps[:], h[:, k * 128:(k + 1) * 128], id_sb[:])
                nc.scalar.copy(hT[:, k, :], tps[:])
            hT_prev = hT

        # Down-projection chunks interleaved with LSTM blocks:
        # chunk c covers output columns [c*CCOLS, (c+1)*CCOLS) = t-block
        steps_per_chunk = T // DP_CHUNKS
        for c in range(DP_CHUNKS):
            ps = dpp.tile([128, CCOLS], mybir.dt.float32, tag="dp")
            for k in range(KT):
                sl = slabs.tile([128, CCOLS], mmdt, tag="slab")
                nc.sync.dma_start(sl[:], xT[k * 128:(k + 1) * 128, c * CCOLS:(c + 1) * CCOLS])
                nc.tensor.matmul(ps[:], Wd_sb[:, k * 128:(k + 1) * 128], sl[:],
                                 start=(k == 0), stop=(k == KT - 1))
            nc.scalar.copy(xaT_sb[c][:], ps[:])
            for t in range(c * steps_per_chunk, (c + 1) * steps_per_chunk):
                lstm_step(t)

        # output head: pred = h_T @ W_out + b_out
        ps_p = tp.tile([BC, 1], mybir.dt.float32, tag="tp")
        for k in range(4):
            nc.tensor.matmul(ps_p[:], hT_prev[:, k, :], Wo_sb[:, k:k + 1],
                             start=(k == 0), stop=(k == 3))
        out_sb = singles.tile([BC, 1], mybir.dt.float32)
        nc.scalar.activation(out_sb[:], ps_p[:], AF.Identity, bias=bout_sb[:])
        nc.sync.dma_start(pred[:], out_sb[:])

    nc.finalize()
    return nc


def _get_module():
    key = MM_DTYPE
    if key not in _CACHE:
        _CACHE[key] = _build_module()
    return _CACHE[key]


def _prep_inputs(x, W_down, W_cell, b_cell, W_out, b_out):
    mmnp = _np_mm_dtype()
    x = np.asarray(x, dtype=np.float32)
    W_down = np.asarray(W_down, dtype=np.float32)
    W_cell = np.asarray(W_cell, dtype=np.float32)
    b_cell = np.asarray(b_cell, dtype=np.float32)
    W_out = np.asarray(W_out, dtype=np.float32)
    b_out = np.asarray(b_out, dtype=np.float32)

    # shared tensors
    Wd_pad = np.zeros((NPAD, HIN), dtype=np.float32)
    Wd_pad[:N] = W_down
    # [NPAD, 128] -> per-k-tile layout [128, KT*128] (col block k = k-tile)
    Wd_host = np.ascontiguousarray(
        Wd_pad.reshape(KT, 128, HIN).transpose(1, 0, 2).reshape(128, KT * HIN)
    ).astype(mmnp)
    Wxa_host = np.ascontiguousarray(W_cell[0:HIN]).astype(mmnp)          # [128, 2048]
    Wxwb_host = np.stack([W_cell[HIN], b_cell]).astype(mmnp)             # [2, 2048]
    Wh_host = np.ascontiguousarray(
        W_cell[HIN + 1:].reshape(4, 128, G4).transpose(1, 0, 2).reshape(128, 4 * G4)
    ).astype(mmnp)                                                       # [128, 4*2048]
    Wo_host = np.ascontiguousarray(W_out.reshape(4, 128).T).astype(mmnp)  # [128, 4]
    bout_host = np.full((BC, 1), float(b_out[0]), dtype=np.float32)
    id_host = np.eye(BC, dtype=np.float32).astype(mmnp)

    in_maps = []
    for i in range(NCORES):
        xs = x[i * BC:(i + 1) * BC]                       # [64, 16, 10001]
        # xT: [NPAD, 1024], column index = t*64 + b (t-major)
        xT_host = np.zeros((NPAD, BT), dtype=mmnp)
        xT_host[:N] = xs[:, :, :N].transpose(2, 1, 0).reshape(N, BT).astype(mmnp)
        xw_host = np.empty((2, BT), dtype=mmnp)
        xw_host[0] = xs[:, :, N].T.reshape(BT).astype(mmnp)
        xw_host[1] = np.ones(BT, dtype=np.float32).astype(mmnp)
        in_maps.append({
            "xT": xT_host,
            "xw": xw_host,
            "Wd": Wd_host,
            "Wxa": Wxa_host,
            "Wxwb": Wxwb_host,
            "Wh": Wh_host,
            "Wo": Wo_host,
            "bout": bout_host,
            "ident": id_host,
        })
    return in_maps


def run(trace=False, **inputs):
    from concourse.bass_utils import run_bass_kernel_spmd

    nc = _get_module()
    in_maps = _prep_inputs(**inputs)
    res = run_bass_kernel_spmd(nc, in_maps, list(range(NCORES)), trace=trace)
    pred = np.concatenate([res.results[i]["pred"] for i in range(NCORES)], axis=0)
    return pred.astype(np.float32), res


def kernel(**inputs):
    pred, _ = run(trace=False, **inputs)
    return pred


# revision 15
# speedup vs baseline: 1.2067x; 1.2067x over previous
"""Trainium2 Bass kernel for nn_Discriminator (down-projection + 16-step LSTM + head).

Computation (per reference):
    x: [512, 16, 10001] fp32
    xa = x[:, :, :10000] @ W_down                      # [B, T, 128]
    xc_t = concat([xa_t, xw_t], -1)                    # per step, [B, 129]
    LSTM over T=16 steps, H=512, forget bias +1:
        gates = [xc_t, h] @ W_cell + b_cell            # [B, 2048] = [i|c|f|o]
        c = c*sig(f+1) + sig(i)*tanh(c_)
        h = sig(o)*tanh(c)
    pred = h @ W_out + b_out                           # [B, 1]

Sharding: pure data-parallel over batch, 64 rows/core on 8 cores. No collectives.

Per-core layout choices:
  - x is pre-transposed on host to xT [10112(pad), 1024] with column index
    t*64+b (t-major) so the PE contraction dim (n) is on partitions, and the
    down-projection directly produces xa^T [128, 1024] whose per-step slice
    [128, 64] is the lhsT of the LSTM's xa matmul.
  - LSTM gates are computed in PSUM [128, 1024]: partitions 0:64 hold the
    [i|c] gate columns, partitions 64:128 hold [f|o]; the two halves are fed
    by concurrent PE column-group matmuls (tile col-tiling), so no extra adds.
  - sigmoid(i) and sigmoid(f+1) are fused into a single [128, 512] activation
    with a per-partition bias of (0 x64, 1 x64).
  - h is transposed back each step via PE-transpose (4x [64,128] tiles) to
    form the next step's lhsT.
"""

import numpy as np
from contextlib import ExitStack

NCORES = 8
B = 512
BC = B // NCORES          # 64 batch rows per core
T = 16
BT = BC * T               # 1024
N = 10000
KT = 79                   # ceil(10000/128)
NPAD = KT * 128           # 10112
HIN = 128
H = 512
G4 = 4 * H                # 2048
DP_CHUNKS = 2             # down-projection output chunks (t-blocks)
CCOLS = BT // DP_CHUNKS   # columns per chunk (512)

# Compute dtype for matmul operands: "float32r" (full-rate fp32 PE mode) or
# "bfloat16" (halves HBM traffic for x; small accuracy cost).
MM_DTYPE = "bfloat16"

_CACHE = {}


def _np_mm_dtype():
    if MM_DTYPE == "bfloat16":
        import ml_dtypes
        return ml_dtypes.bfloat16
    return np.float32


def _build_module():
    import concourse.bass as bass  # noqa: F401
    import concourse.bacc as bacc
    import concourse.tile as tile
    import concourse.mybir as mybir

    AF = mybir.ActivationFunctionType
    f32 = mybir.dt.float32
    mmdt = mybir.dt.bfloat16 if MM_DTYPE == "bfloat16" else mybir.dt.float32r

    nc = bacc.Bacc("TRN2")

    xT = nc.declare_dram_parameter("xT", [NPAD, BT], mmdt, isOutput=False)
    xw = nc.declare_dram_parameter("xw", [2, BT], mmdt, isOutput=False)
    Wd = nc.declare_dram_parameter("Wd", [128, KT * 128], mmdt, isOutput=False)
    Wxa = nc.declare_dram_parameter("Wxa", [128, G4], mmdt, isOutput=False)
    Wxwb = nc.declare_dram_parameter("Wxwb", [2, G4], mmdt, isOutput=False)
    Wh = nc.declare_dram_parameter("Wh", [128, 4 * G4], mmdt, isOutput=False)
    Wo = nc.declare_dram_parameter("Wo", [128, 4], mmdt, isOutput=False)
    bout = nc.declare_dram_parameter("bout", [BC, 1], f32, isOutput=False)
    ident = nc.declare_dram_parameter("ident", [BC, BC], mmdt, isOutput=False)
    pred = nc.declare_dram_parameter("pred", [BC, 1], f32, isOutput=True)

    # slab granularity: KG k-tiles per DMA (bigger transfers, fewer issues)
    KG = 4
    NSLAB = (KT + KG - 1) // KG  # 20 (last slab holds 3 k-tiles)

    with ExitStack() as ctx:
        tc = ctx.enter_context(tile.TileContext(nc))
        singles = ctx.enter_context(tc.tile_pool(name="singles", bufs=1))
        slabs = ctx.enter_context(tc.tile_pool(name="slabs", bufs=8))
        work = ctx.enter_context(tc.tile_pool(name="work", bufs=2))
        state = ctx.enter_context(tc.tile_pool(name="state", bufs=2))
        dpp = ctx.enter_context(tc.tile_pool(name="dpp", bufs=2, space="PSUM"))
        gp = ctx.enter_context(tc.tile_pool(name="gp", bufs=2, space="PSUM"))
        tp = ctx.enter_context(tc.tile_pool(name="tp", bufs=2, space="PSUM"))

        # W_down first (the dp stream's only prerequisite), split across DMA
        # lanes so the first k-tiles land quickly
        Wd_sb = singles.tile([128, KT * 128], mmdt)
        wd_step = 10 * 128
        for o in range(0, KT * 128, wd_step):
            e = min(o + wd_step, KT * 128)
            nc.sync.dma_start(Wd_sb[:, o:e], Wd[:, o:e])

        # xa^T, one tile per chunk so the LSTM's dependency is per-chunk
        xaT_sb = [singles.tile([128, CCOLS], mmdt, name=f"xaT{c}") for c in range(DP_CHUNKS)]

        # down-projection chunk-0 stream emitted first; LSTM weights after
        def dp_chunk(c, ps):
            for s in range(NSLAB):
                k0 = s * KG
                nk = min(KG, KT - k0)
                sl = slabs.tile([128, KG, CCOLS], mmdt, tag="slab")
                src = xT[k0 * 128:(k0 + nk) * 128, c * CCOLS:(c + 1) * CCOLS]
                nc.sync.dma_start(sl[:, :nk, :], src.rearrange("(t p) c -> p t c", p=128))
                for j in range(nk):
                    k = k0 + j
                    nc.tensor.matmul(ps[:], Wd_sb[:, k * 128:(k + 1) * 128], sl[:, j, :],
                                     start=(k == 0), stop=(k == KT - 1))
            nc.scalar.copy(xaT_sb[c][:], ps[:])

        ps0 = dpp.tile([128, CCOLS], mybir.dt.float32, tag="dp")
        dp_chunk(0, ps0)

        # LSTM weights + small tensors (needed only once step 0 starts)
        Wxa_sb = singles.tile([128, G4], mmdt)
        nc.sync.dma_start(Wxa_sb[:], Wxa[:])
        Wxwb_sb = singles.tile([2, G4], mmdt)
        nc.sync.dma_start(Wxwb_sb[:], Wxwb[:])
        Wh_sb = singles.tile([128, 4 * G4], mmdt)
        nc.sync.dma_start(Wh_sb[:], Wh[:])
        Wo_sb = singles.tile([128, 4], mmdt)
        nc.sync.dma_start(Wo_sb[:], Wo[:])
        xw_sb = singles.tile([2, BT], mmdt)
        nc.sync.dma_start(xw_sb[:], xw[:])
        bout_sb = singles.tile([BC, 1], f32)
        nc.sync.dma_start(bout_sb[:], bout[:])
        id_sb = singles.tile([BC, BC], mmdt)
        nc.sync.dma_start(id_sb[:], ident[:])

        hT_prev = None
        c_prev = None
        g_next = None  # psum tile of the NEXT step, pre-accumulated with xa/xw

        def xaxw_mms(t, g, close=False):
            """Accumulate the h-independent gate contributions for step t
            into psum g (start=True). Emitted early so the PE does this work
            while the previous step's activation chain runs. close=True ends
            the accumulation group (used at t=0, which has no h terms)."""
            ktiles = [
                (xaT_sb[t * BC // CCOLS][:, (t * BC) % CCOLS:(t * BC) % CCOLS + BC], Wxa_sb),
                (xw_sb[:, t * BC:(t + 1) * BC], Wxwb_sb),
            ]
            for ki, (lh, rh) in enumerate(ktiles):
                st = ki == 0
                sp = close and ki == len(ktiles) - 1
                for ch in range(2):
                    for half in range(2):
                        outap = g[half * 64:(half + 1) * 64, ch * H:(ch + 1) * H]
                        rhap = rh[:, half * 2 * H + ch * H: half * 2 * H + (ch + 1) * H]
                        nc.tensor.matmul(outap, lh, rhap, start=st, stop=sp)

        def lstm_step(t):
            nonlocal hT_prev, c_prev, g_next
            g = g_next
            # h-dependent gate contributions (the recurrent critical path)
            if hT_prev is not None:
                for k in range(4):
                    lh = hT_prev[:, k, :]
                    rh = Wh_sb[:, k * G4:(k + 1) * G4]
                    sp = k == 3
                    for ch in range(2):
                        for half in range(2):
                            outap = g[half * 64:(half + 1) * 64, ch * H:(ch + 1) * H]
                            rhap = rh[:, half * 2 * H + ch * H: half * 2 * H + (ch + 1) * H]
                            nc.tensor.matmul(outap, lh, rhap, start=False, stop=sp)
            # pre-accumulate next step's h-independent parts while this
            # step's activation chain runs (PE would otherwise idle)
            if t + 1 < T:
                g_next = gp.tile([128, 2 * H], mybir.dt.float32, tag="g")
                xaxw_mms(t + 1, g_next)

            # activations (all outputs are base-partition-0 tiles: the DVE
            # requires matching base partitions on two-SBUF-input ops)
            sig_i = work.tile([BC, H], mybir.dt.float32, tag="si")
            nc.scalar.activation(sig_i[:], g[0:64, 0:H], AF.Sigmoid)
            sig_f = work.tile([BC, H], mybir.dt.float32, tag="sf")
            nc.scalar.activation(sig_f[:], g[64:128, 0:H], AF.Sigmoid, bias=1.0)
            tanh_c = work.tile([BC, H], mybir.dt.float32, tag="tc")
            nc.scalar.activation(tanh_c[:], g[0:64, H:2 * H], AF.Tanh)
            sig_o = work.tile([BC, H], mybir.dt.float32, tag="so")
            nc.scalar.activation(sig_o[:], g[64:128, H:2 * H], AF.Sigmoid)

            c_new = state.tile([BC, H], mybir.dt.float32, tag="c")
            if c_prev is None:
                nc.vector.tensor_mul(c_new[:], sig_i[:], tanh_c[:])
            else:
                m1 = work.tile([BC, H], mybir.dt.float32, tag="m1")
                nc.vector.tensor_mul(m1[:], c_prev[:], sig_f[:])
                m2 = work.tile([BC, H], mybir.dt.float32, tag="m2")
                nc.vector.tensor_mul(m2[:], sig_i[:], tanh_c[:])
                nc.vector.tensor_add(c_new[:], m1[:], m2[:])
            c_prev = c_new

            tanh_cn = work.tile([BC, H], mybir.dt.float32, tag="tcn")
            nc.scalar.activation(tanh_cn[:], c_new[:], AF.Tanh)
            h = work.tile([BC, H], mmdt, tag="h")
            nc.vector.tensor_mul(h[:], sig_o[:], tanh_cn[:])

            # transpose h into the next step's lhsT via 4 PE transposes into
            # one PSUM tile, then a single copy
            hT = state.tile([128, 4, BC], mmdt, tag="hT")
            tps = tp.tile([128, 4 * BC], mmdt, tag="tp")
            for k in range(4):
                nc.tensor.transpose(tps[:, k * BC:(k + 1) * BC],
                                    h[:, k * 128:(k + 1) * 128], id_sb[:])
            nc.scalar.copy(hT[:], tps[:].rearrange("p (k b) -> p k b", k=4))
            hT_prev = hT

        # chunk 0's LSTM block; dp chunk 1 stream is emitted after step 0 so
        # its DMAs/mms fill LSTM idle, then the remaining steps
        steps_per_chunk = T // DP_CHUNKS
        g_next = gp.tile([128, 2 * H], mybir.dt.float32, tag="g")
        xaxw_mms(0, g_next, close=True)
        lstm_step(0)
        ps1 = dpp.tile([128, CCOLS], mybir.dt.float32, tag="dp")
        dp_chunk(1, ps1)
        for t in range(1, T):
            lstm_step(t)

        # output head: pred = h_T @ W_out + b_out
        ps_p = tp.tile([BC, 1], mybir.dt.float32, tag="tp")
        for k in range(4):
            nc.tensor.matmul(ps_p[:], hT_prev[:, k, :], Wo_sb[:, k:k + 1],
                             start=(k == 0), stop=(k == 3))
        out_sb = singles.tile([BC, 1], mybir.dt.float32)
        nc.scalar.activation(out_sb[:], ps_p[:], AF.Identity, bias=bout_sb[:])
        nc.sync.dma_start(pred[:], out_sb[:])

    nc.finalize()
    return nc


def _get_module():
    key = MM_DTYPE
    if key not in _CACHE:
        _CACHE[key] = _build_module()
    return _CACHE[key]


def _prep_inputs(x, W_down, W_cell, b_cell, W_out, b_out):
    mmnp = _np_mm_dtype()
    x = np.asarray(x, dtype=np.float32)
    W_down = np.asarray(W_down, dtype=np.float32)
    W_cell = np.asarray(W_cell, dtype=np.float32)
    b_cell = np.asarray(b_cell, dtype=np.float32)
    W_out = np.asarray(W_out, dtype=np.float32)
    b_out = np.asarray(b_out, dtype=np.float32)

    # shared tensors
    Wd_pad = np.zeros((NPAD, HIN), dtype=np.float32)
    Wd_pad[:N] = W_down
    # [NPAD, 128] -> per-k-tile layout [128, KT*128] (col block k = k-tile)
    Wd_host = np.ascontiguousarray(
        Wd_pad.reshape(KT, 128, HIN).transpose(1, 0, 2).reshape(128, KT * HIN)
    ).astype(mmnp)
    Wxa_host = np.ascontiguousarray(W_cell[0:HIN]).astype(mmnp)          # [128, 2048]
    Wxwb_host = np.stack([W_cell[HIN], b_cell]).astype(mmnp)             # [2, 2048]
    Wh_host = np.ascontiguousarray(
        W_cell[HIN + 1:].reshape(4, 128, G4).transpose(1, 0, 2).reshape(128, 4 * G4)
    ).astype(mmnp)                                                       # [128, 4*2048]
    Wo_host = np.ascontiguousarray(W_out.reshape(4, 128).T).astype(mmnp)  # [128, 4]
    bout_host = np.full((BC, 1), float(b_out[0]), dtype=np.float32)
    id_host = np.eye(BC, dtype=np.float32).astype(mmnp)

    in_maps = []
    for i in range(NCORES):
        xs = x[i * BC:(i + 1) * BC]                       # [64, 16, 10001]
        # xT: [NPAD, 1024], column index = t*64 + b (t-major)
        xT_host = np.zeros((NPAD, BT), dtype=mmnp)
        xT_host[:N] = xs[:, :, :N].transpose(2, 1, 0).reshape(N, BT).astype(mmnp)
        xw_host = np.empty((2, BT), dtype=mmnp)
        xw_host[0] = xs[:, :, N].T.reshape(BT).astype(mmnp)
        xw_host[1] = np.ones(BT, dtype=np.float32).astype(mmnp)
        in_maps.append({
            "xT": xT_host,
            "xw": xw_host,
            "Wd": Wd_host,
            "Wxa": Wxa_host,
            "Wxwb": Wxwb_host,
            "Wh": Wh_host,
            "Wo": Wo_host,
            "bout": bout_host,
            "ident": id_host,
        })
    return in_maps


def run(trace=False, **inputs):
    from concourse.bass_utils import run_bass_kernel_spmd

    nc = _get_module()
    in_maps = _prep_inputs(**inputs)
    res = run_bass_kernel_spmd(nc, in_maps, list(range(NCORES)), trace=trace)
    pred = np.concatenate([res.results[i]["pred"] for i in range(NCORES)], axis=0)
    return pred.astype(np.float32), res


def kernel(**inputs):
    pred, _ = run(trace=False, **inputs)
    return pred
